# revision 1
# baseline (speedup 1.0000x reference)
"""NT-Xent / SimCLR contrastive loss on 8 Trainium2 NeuronCores.

Math (reference):
  z = concat(proj_1, proj_2)            # [2N, D], 2N=8192, D=128
  zn = z / ||z||                        # row L2-normalize
  sim = zn @ zn.T                       # [2N, 2N]
  denom_i   = sum_{j != i} exp(sim_ij / T)
  pos_i     = sim[i, (i+N) mod 2N]
  loss      = mean_i( log(denom_i) - pos_i / T )

Device decomposition (row-parallel over 8 cores, core c owns rows
[c*1024, (c+1)*1024)). Inputs per core: full z natural-packed
[128, 64, 128] (z[128t+p, d] at [p,t,d]) for row norms, full raw z^T
[128(d), 8192(sample)] for the GEMM moving operand, and the core's own
1024 columns of raw z^T for the stationary.

Normalization never materializes zn in natural layout: compact
1/||row|| ([128, nt] via square -> reduce -> ACT Sqrt -> DVE recip) is
re-ordered to sample order with one padded 128x128 DMA-transpose +
reshape DMA, bounced through DRAM, DMA-broadcast to all 128 partitions,
and applied with a single elementwise multiply against raw z^T. Four
independent 2048-column pipelines overlap DMA/DVE/ACT/transpose so the
GEMM starts ~10us in.

Gram phase: 128 matmuls (K=128, N=512, fp32 PSUM); exp + row-sum fused
on ScalarE via activation(Exp, scale=1/T, accum_out=...); exp values
are discarded. denom = rowsum - e^2 (self-sim), log on ScalarE.
positives: sum_i pos_i = 2*<Zn1,Zn2> elementwise, no diagonal
extraction. Host: loss = (sum log(denom) - (2/T)*<Zn1,Zn2>) / 8192.
"""

import numpy as np

P = 128          # partitions / feature dim
NS = 8192        # total samples (2N)
D = 128          # feature dim
NCORES = 8
RB = NS // NCORES    # 1024 rows per core
MT = RB // P         # 8 m-tiles per core
NT = NS // P         # 64 sample tiles
TEMP = 0.5
INV_T = 1.0 / TEMP   # 2.0
NFREE = 512          # matmul moving free dim (fp32 PSUM: one bank)
ACT_CHUNK = 2048     # ScalarE exp chunk (4 PSUM banks)
NCHUNK = NS // ACT_CHUNK   # 4 column chunks
CT = ACT_CHUNK // P        # 16 sample tiles per chunk

_CACHE = {}


def _ensure_paths():
    import sys
    for p in ("/root/.axon_site", "/root/.axon_site/_ro/trn_rl_repo",
              "/root/.axon_site/_ro/pypackages", "/opt/trn_rl_repo", "/opt/pypackages"):
        if p not in sys.path:
            sys.path.append(p)


def _build():
    _ensure_paths()
    import concourse.bass as bass
    import concourse.bacc as bacc
    import concourse.mybir as mybir
    import concourse.tile as tile

    dt_bf = mybir.dt.bfloat16
    dt_f32 = mybir.dt.float32
    AFT = mybir.ActivationFunctionType
    AX = mybir.AxisListType

    nc = bacc.Bacc("TRN2", target_bir_lowering=False, debug=False,
                   num_devices=NCORES)

    znat_d = nc.dram_tensor("znat", [P, NT, P], dt_bf, kind="ExternalInput")
    zt_d = nc.dram_tensor("zt", [P, NS], dt_bf, kind="ExternalInput")
    ld_d = nc.dram_tensor("out_ld", [P, MT], dt_f32, kind="ExternalOutput")
    pos_d = nc.dram_tensor("out_pos", [P, 1], dt_f32, kind="ExternalOutput")
    ninv_dram = nc.dram_tensor("ninv_row_scratch", [1, NS], dt_bf)

    with tile.TileContext(nc) as tc:
        with (
            tc.tile_pool(name="big", bufs=1) as big,
            tc.tile_pool(name="work", bufs=2) as work,
            tc.tile_pool(name="psum", bufs=2, space=bass.MemorySpace.PSUM) as psum,
        ):
            zt = big.tile([P, NS], dt_bf, tag="zt")        # z^T raw
            zn_t = big.tile([P, NS], dt_bf, tag="zn_t")    # zn^T
            rs = big.tile([P, MT * NCHUNK], dt_f32, tag="rs")

            # ---- sqrt-table preload: Square and Sqrt both live in the
            # sqrt_and_others table, so a dummy Sqrt issued before the
            # first Square pins that table while the input DMAs stream
            # (4 table loads -> 3, first one hidden under DMA) ----
            scr = big.tile([P, 1], dt_f32, tag="scr")
            nc.gpsimd.memset(scr[:], 1.0)
            scr2 = big.tile([P, 1], dt_f32, tag="scr2")
            nc.scalar.activation(scr2[:], scr[:], AFT.Sqrt)

            # ---- bulk loads (inputs are pre-rolled per core so own
            # rows are always samples 0..1023: the gram stationary is
            # zn_t[:, :1024] and no separate own-rows pipeline exists) ----
            zna = big.tile([P, NT, P], dt_bf, tag="zna")
            for h in range(NCHUNK):
                nc.sync.dma_start(zna[:, h * CT:(h + 1) * CT, :],
                                  znat_d[:, h * CT:(h + 1) * CT, :])
                nc.sync.dma_start(zt[:, h * ACT_CHUNK:(h + 1) * ACT_CHUNK],
                                  zt_d[:, h * ACT_CHUNK:(h + 1) * ACT_CHUNK])

            # ---- full-z norms + normalize, 4 independent chunks ----
            ss = big.tile([P, NT], dt_f32, tag="ss")
            sn = big.tile([P, NT], dt_f32, tag="sn")
            ninvf = big.tile([P, NT], dt_f32, tag="ninvf")
            ninv_b = big.tile([P, NS], dt_bf, tag="ninv_b")
            for h in range(NCHUNK):
                tsl = slice(h * CT, (h + 1) * CT)          # tile indices
                csl = slice(h * ACT_CHUNK, (h + 1) * ACT_CHUNK)  # columns
                sqg = work.tile([P, CT, P], dt_bf, tag="sq")
                nc.scalar.activation(sqg[:], zna[:, tsl, :], AFT.Square)
                nc.vector.reduce_sum(ss[:, tsl], sqg[:], axis=AX.X)
                nc.scalar.activation(sn[:, tsl], ss[:, tsl], AFT.Sqrt)
                nc.vector.reciprocal(ninvf[:, tsl], sn[:, tsl])
                nvb = work.tile([P, P], dt_bf, tag="nvb")  # cols CT: pad
                nc.gpsimd.memset(nvb[:, CT:], 1.0)
                nc.vector.tensor_copy(nvb[:, :CT], ninvf[:, tsl])
                nvt = work.tile([P, P], dt_bf, tag="nvt")
                nc.sync.dma_start_transpose(nvt[:], nvb[:])
                nc.gpsimd.dma_start(ninv_dram[0:1, csl], nvt[0:CT, :])
                nc.gpsimd.dma_start(ninv_b[:, csl],
                                    ninv_dram[0:1, csl].broadcast_to(
                                        [P, ACT_CHUNK]))
            for h in range(NCHUNK):
                csl = slice(h * ACT_CHUNK, (h + 1) * ACT_CHUNK)
                nc.vector.tensor_mul(zn_t[:, csl], zt[:, csl],
                                     ninv_b[:, csl])

            # ---- positives: <Zn1, Zn2>; emitted before the gram loop
            # so the DVE work overlaps the exp phase instead of
            # serializing into the tail ----
            pp = big.tile([P, NS // 2], dt_bf, tag="pp")
            nc.vector.tensor_mul(pp[:], zn_t[:, :NS // 2], zn_t[:, NS // 2:])
            posb = big.tile([P, 1], dt_f32, tag="posb")
            nc.vector.reduce_sum(posb[:], pp[:], axis=AX.X)
            nc.gpsimd.dma_start(pos_d[:], posb[:])

            # exp-table preload: runs as soon as the last Sqrt retires,
            # overlapping the first gram matmuls
            nc.scalar.activation(scr2[:], scr[:], AFT.Exp)

            # ---- gram rows + fused exp/row-sum ----
            nmm = ACT_CHUNK // NFREE               # 4 matmuls per chunk
            for q in range(NCHUNK):
                for m in range(MT):
                    ps = psum.tile([P, ACT_CHUNK], dt_f32, tag="ps")
                    for j in range(nmm):
                        c0 = q * ACT_CHUNK + j * NFREE
                        nc.tensor.matmul(ps[:, j * NFREE:(j + 1) * NFREE],
                                         zn_t[:, m * P:(m + 1) * P],
                                         zn_t[:, c0:c0 + NFREE],
                                         start=True, stop=True)
                    esc = work.tile([P, ACT_CHUNK], dt_bf, tag="esc")
                    idx = q * MT + m
                    nc.scalar.activation(esc[:], ps[:], AFT.Exp, scale=INV_T,
                                         accum_out=rs[:, idx:idx + 1])

            # ---- denominators -> log ----
            rsum = big.tile([P, MT], dt_f32, tag="rsum")
            nc.vector.reduce_sum(rsum[:], rs[:].rearrange("p (q m) -> p m q",
                                                          m=MT), axis=AX.X)
            den = big.tile([P, MT], dt_f32, tag="den")
            nc.vector.tensor_scalar_add(den[:], rsum[:],
                                        -float(np.exp(2.0)))
            ldb = big.tile([P, MT], dt_f32, tag="ldb")
            nc.scalar.activation(ldb[:], den[:], AFT.Ln)
            nc.gpsimd.dma_start(ld_d[:], ldb[:])

    nc.compile()
    return nc


def get_nc():
    if "nc" not in _CACHE:
        _CACHE["nc"] = _build()
    return _CACHE["nc"]


def make_in_maps(proj_1: np.ndarray, proj_2: np.ndarray):
    import ml_dtypes
    z = np.concatenate([np.asarray(proj_1), np.asarray(proj_2)], axis=0)
    zb = z.astype(ml_dtypes.bfloat16)
    in_maps = []
    for c in range(NCORES):
        # roll so core c's own rows are samples 0..1023; row sums and
        # the +4096 positives pairing are invariant under the roll
        zc = np.roll(zb, -RB * c, axis=0)
        znat = np.ascontiguousarray(zc.reshape(NT, P, P).transpose(1, 0, 2))
        ztr = np.ascontiguousarray(zc.T)
        in_maps.append({"znat": znat, "zt": ztr})
    return in_maps


def finish(results) -> np.ndarray:
    ld_sum = 0.0
    pos_vals = []
    for r in results:
        ld_sum += float(np.asarray(r["out_ld"], dtype=np.float64).sum())
        pos_vals.append(float(np.asarray(r["out_pos"], dtype=np.float64).sum()))
    pos_dot = float(np.mean(pos_vals))
    loss = (ld_sum - 2.0 * INV_T * pos_dot) / float(NS)
    return np.float32(loss)


def kernel(proj_1: np.ndarray, proj_2: np.ndarray) -> np.ndarray:
    _ensure_paths()
    from concourse.bass_utils import run_bass_kernel_spmd
    nc = get_nc()
    in_maps = make_in_maps(proj_1, proj_2)
    res = run_bass_kernel_spmd(nc, in_maps, core_ids=list(range(NCORES)))
    return finish(res.results)



# revision 2
# speedup vs baseline: 1.0534x; 1.0534x over previous
"""NT-Xent / SimCLR contrastive loss on 8 Trainium2 NeuronCores, v2.

Exploits gram symmetry: each unordered 128x128 tile pair of the 64x64
tile grid is exp'd ONCE (2080 tiles vs 4096), halving ScalarE exp work.
Core c (input rolled by -128c samples) computes local row-tiles
L in {0,8,...,56} against col-tiles (L+d) mod 64, d=0..31, plus d=32
tiles for L in {0,8,16,24}. Row sums via exp accum_out; transpose-side
column sums accumulate tile-wise into a [128, 8192] bf16 SBUF
accumulator (DVE adds; fully-virgin chunks of rows 0/32 write exp
output straight into the accumulator). Per 512-col region, once its
last covering row-tile is done, a ones-stationary matmul reduces the
128 partitions into a dedicated PSUM bank (4 regions per bank at
partition offsets 0/32/64/96) and a grouped DVE copy moves 4 regions
at once to SBUF for DMA-out. Positives are computed exactly as
elementwise zn-block products (DVE) summed per-feature; host sums the
rest. Host combines partial row/col sums across cores, subtracts the
self-sim e^2, and takes log — the cross-core reduction the sharding
hint assigns to the final gather.

Chunk layout per row-tile: A = d-cols 128..1663, B = 1664..3199,
C = 3200..4095 + the deferred d0 block (+ d32 block). The d0 block
never enters the accumulator (its transpose half is its own row sum).
"""

import numpy as np

P = 128
NS = 8192            # 2N
D = 128
NCORES = 8
NT = NS // P         # 64 col tiles
TEMP = 0.5
INV_T = 1.0 / TEMP
ORDER = [0, 32, 8, 40, 16, 48, 24, 56]  # processing order of row-tiles
D32ROWS = (0, 8, 16, 24)
NCH = 4              # norm chunks
CHW = NS // NCH      # 2048
CT = CHW // P        # 16
PSW = 1536           # gram PSUM chunk width (3 banks)
NREG = 16
REGW = NS // NREG

_CACHE = {}


def _ensure_paths():
    import sys
    for p in ("/root/.axon_site", "/root/.axon_site/_ro/trn_rl_repo",
              "/root/.axon_site/_ro/pypackages", "/opt/trn_rl_repo", "/opt/pypackages"):
        if p not in sys.path:
            sys.path.append(p)


def _mm_segs(L, pieces):
    """Split k-pieces into <=512-wide matmul segments that never cross
    the mod-8192 wrap. pieces: list of (k0, k1) over the row's d-cols,
    or ("abs", col, w) for absolute column blocks."""
    segs = []
    off = 0
    for pc in pieces:
        if pc[0] == "abs":
            _, c0, w = pc
            segs.append((off, c0, w))
            off += w
            continue
        k0, k1 = pc
        k = k0
        while k < k1:
            c = (128 * L + k) % NS
            # cap at the PSUM bank boundary (512 f32) and the mod-NS wrap
            w = min(512 - (off % 512), k1 - k, NS - c)
            segs.append((off, c, w))
            off += w
            k += w
    return segs, off


def _plan():
    """Static schedule. Returns list of steps, each:
    L, width, segs [(psum_off, col, w)], target ("acc"|"esc"),
    adds [(esc_off, dst_col, w, is_copy)], regions [r...]."""
    steps = []
    written = np.zeros(NS, dtype=bool)
    for L in ORDER:
        chunks = [[(128, 1664)], [(1664, 3200)]]
        cpieces = [(3200, 4096), (0, 128)]          # k-tail + deferred d0
        chunks.append(cpieces)
        for ci, pieces in enumerate(chunks):
            is_c = ci == 2
            if is_c and L in D32ROWS:
                pieces = pieces + [("abs", (L + 32) * 128, 128)]
            segs, width = _mm_segs(L, pieces)
            # acc-target cols for this chunk: all except the d0 piece
            acc_runs = []   # (esc_off, dst_col, w) contiguous in dst
            off = 0
            for pc in pieces:
                if pc[0] == "abs":
                    acc_runs.append((off, pc[1], pc[2]))
                    off += pc[2]
                    continue
                k0, k1 = pc
                if (k0, k1) == (0, 128):   # d0: skip acc entirely
                    off += 128
                    continue
                k = k0
                while k < k1:
                    c = (128 * L + k) % NS
                    w = min(k1 - k, NS - c)
                    acc_runs.append((off, c, w))
                    off += w
                    k += w
            virgin = all(not written[c:c + w].any() for _, c, w in acc_runs)
            direct = virgin and not is_c and len(acc_runs) == 1
            adds = []
            if not direct:
                for eoff, c, w in acc_runs:
                    i = 0
                    while i < w:
                        v = bool(written[c + i])
                        j = i
                        while j < w and bool(written[c + j]) == v:
                            j += 1
                        adds.append((eoff + i, c + i, j - i, not v))
                        i = j
            for _, c, w in acc_runs:
                written[c:c + w] = True
            steps.append(dict(L=L, width=width, segs=segs,
                              target="acc" if direct else "esc",
                              acc_col=acc_runs[0][1] if direct else None,
                              adds=adds))
    assert written.all()
    reg_last = [-1] * NREG
    for si, st in enumerate(steps):
        wr = ([(st["acc_col"], st["width"])] if st["target"] == "acc"
              else [(c, w) for _, c, w, _ in st["adds"]])
        for c, w in wr:
            for r in range(c // REGW, (c + w - 1) // REGW + 1):
                reg_last[r] = si
    order = []
    for si, st in enumerate(steps):
        st["regions"] = [r for r in range(NREG) if reg_last[r] == si]
        order.extend(st["regions"])
    assert len(order) == NREG
    return steps


def _build():
    _ensure_paths()
    import concourse.bass as bass
    import concourse.bacc as bacc
    import concourse.mybir as mybir
    import concourse.tile as tile

    dt_bf = mybir.dt.bfloat16
    dt_f32 = mybir.dt.float32
    AFT = mybir.ActivationFunctionType
    AX = mybir.AxisListType
    ALU = mybir.AluOpType

    steps = _plan()

    nc = bacc.Bacc("TRN2", target_bir_lowering=False, debug=False,
                   num_devices=NCORES)

    znat_d = nc.dram_tensor("znat", [P, NT, P], dt_bf, kind="ExternalInput")
    zt_d = nc.dram_tensor("zt", [P, NS], dt_bf, kind="ExternalInput")
    rs_d = nc.dram_tensor("out_rs", [P, 24], dt_f32, kind="ExternalOutput")
    cs_d = nc.dram_tensor("out_cs", [4, 4, REGW], dt_f32, kind="ExternalOutput")
    pos_d = nc.dram_tensor("out_pos", [P, 4], dt_f32, kind="ExternalOutput")
    rrow_dram = nc.dram_tensor("rrow_scratch", [1, NS], dt_bf)

    with tile.TileContext(nc) as tc:
        with (
            tc.tile_pool(name="big", bufs=1) as big,
            tc.tile_pool(name="work", bufs=2) as work,
            tc.tile_pool(name="psum", bufs=2, space=bass.MemorySpace.PSUM) as psum,
        ):
            zt = big.tile([P, NS], dt_bf, tag="zt")
            zna = big.tile([P, NT, P], dt_bf, tag="zna")
            zn = big.tile([P, NS], dt_bf, tag="zn")
            rb = big.tile([P, NS], dt_bf, tag="rb")
            acc = big.tile([P, NS], dt_bf, tag="acc")
            ss = big.tile([P, NT], dt_f32, tag="ss")
            sn = big.tile([P, NT], dt_f32, tag="sn")
            rf = big.tile([P, NT], dt_f32, tag="rf")
            rs = big.tile([P, 24], dt_f32, tag="rs")
            posb = big.tile([P, 4], dt_f32, tag="posb")
            ones = big.tile([P, 32], dt_bf, tag="ones")
            cs_sb = big.tile([P, 2 * REGW], dt_f32, tag="cs_sb")
            nvb = [big.tile([P, P], dt_bf, tag=f"nvb{i}", name=f"nvb{i}")
                   for i in range(2)]
            scr = big.tile([P, 1], dt_f32, tag="scr")
            scr2 = big.tile([P, 1], dt_f32, tag="scr2")

            # sqrt-table preload under the input DMAs
            nc.gpsimd.memset(scr[:], 1.0)
            nc.scalar.activation(scr2[:], scr[:], AFT.Sqrt)
            nc.gpsimd.memset(ones[:], 1.0)
            for t in nvb:
                nc.gpsimd.memset(t[:], 1.0)

            for h in range(NCH):
                nc.sync.dma_start(zna[:, h * CT:(h + 1) * CT, :],
                                  znat_d[:, h * CT:(h + 1) * CT, :])
            for h in range(2):
                nc.sync.dma_start(zt[:, h * NS // 2:(h + 1) * NS // 2],
                                  zt_d[:, h * NS // 2:(h + 1) * NS // 2])

            # ---- norms ----
            for h in range(NCH):
                tsl = slice(h * CT, (h + 1) * CT)
                csl = slice(h * CHW, (h + 1) * CHW)
                prio = tc.high_priority() if h == 0 else None
                if prio is not None:
                    prio.__enter__()
                sq = work.tile([P, CT, P], dt_bf, tag="sq")
                nc.vector.tensor_tensor(sq[:], zna[:, tsl, :], zna[:, tsl, :],
                                        ALU.mult)
                sqh = work.tile([P, CT, P // 2], dt_bf, tag="sqh")
                nc.vector.tensor_tensor(sqh[:], sq[:, :, 0:64],
                                        sq[:, :, 64:128], ALU.add)
                nc.vector.reduce_sum(ss[:, tsl], sqh[:], axis=AX.X)
                nc.scalar.activation(sn[:, tsl], ss[:, tsl], AFT.Sqrt)
                nv = nvb[h % 2]
                with nc.allow_low_precision("bf16 zn path tolerates it"):
                    nc.vector.reciprocal(nv[:, 0:CT], sn[:, tsl])
                nvt = work.tile([P, P], dt_bf, tag="nvt", bufs=4)
                nc.scalar.dma_start_transpose(nvt[:], nv[:])
                nc.scalar.dma_start(rrow_dram[0:1, csl], nvt[0:CT, :])
                nc.gpsimd.dma_start(rb[:, csl],
                                    rrow_dram[0:1, csl].broadcast_to([P, CHW]))
                if prio is not None:
                    prio.__exit__(None, None, None)
            for h in range(NCH):
                csl = slice(h * CHW, (h + 1) * CHW)
                nc.vector.tensor_tensor(zn[:, csl], zt[:, csl], rb[:, csl],
                                        ALU.mult)

            # exp-table preload; input depends on the last sqrt so the
            # scheduler cannot hoist it before the sqrts (table thrash)
            nc.scalar.activation(scr2[:], sn[:, NT - 1:NT], AFT.Exp)

            # ---- gram / exp / colsum ----
            d32k = 0
            csg = 0          # colsum group counter (4 regions per group)
            cs_meta = []     # (region, group, slot)
            cs_ps = None
            for rcol, st in enumerate(steps):
                L = st["L"]
                stat = zn[:, 128 * L:128 * L + 128]
                W = st["width"]
                ps = psum.tile([P, PSW], dt_f32, tag="ps")
                for off, col, w in st["segs"]:
                    nc.tensor.matmul(ps[:, off:off + w], stat,
                                     zn[:, col:col + w],
                                     start=True, stop=True)
                if st["target"] == "acc":
                    c0 = st["acc_col"]
                    nc.scalar.activation(acc[:, c0:c0 + W], ps[:, 0:W],
                                         AFT.Exp, scale=INV_T,
                                         accum_out=rs[:, rcol:rcol + 1])
                else:
                    esc = work.tile([P, PSW], dt_bf, tag="esc", bufs=3)
                    nc.scalar.activation(esc[:, 0:W], ps[:, 0:W], AFT.Exp,
                                         scale=INV_T,
                                         accum_out=rs[:, rcol:rcol + 1])
                    for eoff, c, w, is_copy in st["adds"]:
                        if is_copy:
                            nc.vector.tensor_copy(acc[:, c:c + w],
                                                  esc[:, eoff:eoff + w])
                        else:
                            nc.vector.tensor_tensor(acc[:, c:c + w],
                                                    acc[:, c:c + w],
                                                    esc[:, eoff:eoff + w],
                                                    ALU.add)
                # positives for the d32 rows, from zn directly
                if st["L"] in D32ROWS and st["segs"][-1][1] == (L + 32) * 128:
                    a, b = 128 * L, (L + 32) * 128
                    ppd = work.tile([P, P], dt_bf, tag="ppd")
                    nc.vector.tensor_tensor(ppd[:], zn[:, a:a + 128],
                                            zn[:, b:b + 128], ALU.mult)
                    nc.vector.reduce_sum(posb[:, d32k:d32k + 1], ppd[:],
                                         axis=AX.X)
                    d32k += 1
                # colsum region reduction via ones-matmul
                for r in st["regions"]:
                    slot = len(cs_meta) % 4
                    if slot == 0:
                        cs_ps = psum.tile([P, REGW], dt_f32, tag="csps",
                                          name="cs_ps")
                    nc.tensor.matmul(cs_ps[32 * slot:32 * slot + 32, :],
                                     ones[:], acc[:, r * REGW:(r + 1) * REGW],
                                     start=True, stop=True,
                                     tile_position=(0, 32 * slot))
                    cs_meta.append((r, csg, slot))
                    if slot == 3:
                        gcol = (csg % 2) * REGW
                        nc.vector.tensor_copy(
                            cs_sb[:, gcol:gcol + REGW], cs_ps[:])
                        nc.gpsimd.dma_start(
                            cs_d[csg, :, :],
                            cs_sb[0:97:32, gcol:gcol + REGW])
                        csg += 1

            nc.gpsimd.dma_start(rs_d[:], rs[:])
            nc.gpsimd.dma_start(pos_d[:], posb[:])

    nc.compile()
    return nc


def get_nc():
    if "nc" not in _CACHE:
        _CACHE["nc"] = _build()
    return _CACHE["nc"]


def make_in_maps(proj_1: np.ndarray, proj_2: np.ndarray):
    import ml_dtypes
    z = np.concatenate([np.asarray(proj_1), np.asarray(proj_2)], axis=0)
    zb = z.astype(ml_dtypes.bfloat16)
    in_maps = []
    for c in range(NCORES):
        zc = np.roll(zb, -P * c, axis=0)
        znat = np.ascontiguousarray(zc.reshape(NT, P, P).transpose(1, 0, 2))
        ztr = np.ascontiguousarray(zc.T)
        in_maps.append({"znat": znat, "zt": ztr})
    return in_maps


def finish(results) -> np.ndarray:
    steps = _plan()
    reg_order = [r for st in steps for r in st["regions"]]
    denom = np.zeros(NS, dtype=np.float64)
    pos = 0.0
    for c, r in enumerate(results):
        rsv = np.asarray(r["out_rs"], dtype=np.float64)
        csg = np.asarray(r["out_cs"], dtype=np.float64).reshape(NREG, REGW)
        local = np.zeros(NS)
        for i, reg in enumerate(reg_order):
            local[reg * REGW:(reg + 1) * REGW] = csg[i]
        for rcol, st in enumerate(steps):
            L = st["L"]
            local[128 * L:128 * L + 128] += rsv[:, rcol]
        denom += np.roll(local, P * c)
        pos += float(np.asarray(r["out_pos"], dtype=np.float64).sum())
    denom -= np.exp(2.0)
    loss = (np.log(denom).sum() - 2.0 * INV_T * pos) / float(NS)
    return np.float32(loss)


def kernel(proj_1: np.ndarray, proj_2: np.ndarray) -> np.ndarray:
    _ensure_paths()
    from concourse.bass_utils import run_bass_kernel_spmd
    nc = get_nc()
    in_maps = make_in_maps(proj_1, proj_2)
    res = run_bass_kernel_spmd(nc, in_maps, core_ids=list(range(NCORES)))
    return finish(res.results)
